# revision 3
# baseline (speedup 1.0000x reference)
"""Trainium2 Bass kernel for nn_ContextGatingSigmoidClassifier.

Math (eval mode):
  f_so = lrelu(W_so @ bn_so(x_so) + b_so)        x: [B,2048,N,H,W]
  f_c  = lrelu(W_c  @ bn_c(x_c)  + b_c)
  f    = concat -> bn1 -> W1 -> bn2 -> lrelu -> W2 -> mean(H,W) -> sigmoid > 0.5

All BatchNorms are eval-mode affine maps, so they fold into the adjacent
linear layers (done host-side in fp64). Final threshold:
  sigmoid(mean) > 0.5  <=>  sum_hw(W2 @ h) > -49*b2.

Device mapping: data-parallel over batch (4 per core, 8 cores), weights
replicated. x is cast fp32->fp16 host-side so the device reads half the
bytes; the kernel is tensor-engine-bound (1168 matmuls of N=294 ~146us),
so all DMA must hide under the PE stream:
  - x rides the sync HWDGE ring (FIFO, in consumption order, 4 sub-DMAs
    per tensor so the first matmuls start as soon as the first chunks land)
  - weights ride the scalar HWDGE ring (wso split into o-blocks so the
    first accumulation group only waits for 0.5MB)
  - tiny tensors (bias pack, w2) ride the gpsimd SWDGE ring
  - a burst of dummy matmuls at t~7us pre-warms the PE HAM clock gate
    while the first DMAs are still in flight.
Per batch element: x[b] is [2048, 588] (channels on SBUF partitions in
K-chunks of 128, positions on the free dim, 2 tiles of 294 = one PSUM
bank each). Channel->partition mapping is interleaved (partition p holds
channels 16p..16p+15) so every DMA descriptor is one contiguous
per-partition run; weights are permuted host-side to match.
"""

import numpy as np

import concourse.bass as bass  # noqa: F401
import concourse.tile as tile
from concourse import bacc, mybir
from concourse.bass_utils import run_bass_kernel_spmd

F16 = mybir.dt.float16
F32 = mybir.dt.float32

B, C, NN, HW = 32, 2048, 12, 49
NHW = NN * HW            # 588
N_CORES = 8
BPC = B // N_CORES       # 4 batch elements per core
MT = NHW // 2            # 294 columns = one PSUM bank of fp32
KC1 = C // 128           # 16 K-chunks, layer 1
OC1 = 512 // 128         # 4 output chunks, layer 1 (per branch)
KC2 = 1024 // 128        # 8 K-chunks, layer 2
OC2 = 256 // 128         # 2 output chunks, layer 2
SUB = 4                  # x sub-DMAs per tensor (4 k-chunks each)
EPS = 1e-5
SLOPE = 0.2


def _fold_params(d):
    """Fold BNs into linears, in fp64. Returns device-layout arrays."""
    g = {k: np.asarray(v, dtype=np.float64) for k, v in d.items()}

    def bn_st(p):
        s = g[f"{p}_g"] / np.sqrt(g[f"{p}_v"] + EPS)
        t = g[f"{p}_b"] - g[f"{p}_m"] * s
        return s, t

    s_so, t_so = bn_st("bn_so")
    s_c, t_c = bn_st("bn_c")
    s1, t1 = bn_st("bn1")
    s2, t2 = bn_st("bn2")

    A_so = g["W_so"] * s_so[None, :]                 # [512, 2048]
    a_so = g["W_so"] @ t_so + g["b_so"]              # [512]
    A_c = g["W_c"] * s_c[None, :]
    a_c = g["W_c"] @ t_c + g["b_c"]
    A1 = s2[:, None] * (g["W1"] * s1[None, :])       # [256, 1024]
    a1 = s2 * (g["W1"] @ t1 + g["b1"]) + t2          # [256]

    # layer-1 weights, o-major: W_dev[p, o, k, m] = A[128o+m, 16p+k]
    # (chunk k holds channel 16p+k at partition p, matching the x DMA
    # layout; o-major so the first output block is one contiguous DMA).
    def l1_prep(A):  # [512, 2048] -> [128, OC1*KC1*128] fp16
        A4 = A.reshape(OC1, 128, 128, KC1)           # [o, m, p, k]
        return np.ascontiguousarray(
            A4.transpose(2, 0, 3, 1).reshape(128, OC1 * KC1 * 128)
        ).astype(np.float16)

    wso = l1_prep(A_so)
    wc = l1_prep(A_c)
    # layer-2: W1_dev[p, o, k, m] = A1[128o+m, 128k+p] (f stores channel
    # 128k+p at partition p of column-block k).
    A4 = A1.reshape(OC2, 128, KC2, 128)              # [o, m, k, p]
    w1 = np.ascontiguousarray(
        A4.transpose(3, 0, 2, 1).reshape(128, OC2 * KC2 * 128)
    ).astype(np.float16)
    w2 = np.ascontiguousarray(g["W2"].reshape(OC2, 128).T).astype(np.float16)
    # bias pack [128, 10] fp32: bso(4) | bc(4) | b1(2)
    bias = np.concatenate([
        a_so.reshape(OC1, 128).T, a_c.reshape(OC1, 128).T,
        a1.reshape(OC2, 128).T], axis=1)
    bias = np.ascontiguousarray(bias).astype(np.float32)
    thresh = float(-HW * g["b2"][0])
    return wso, wc, w1, w2, bias, thresh


def build_bass(thresh, repeat=1, loop=1):
    nc = bacc.Bacc("TRN2", target_bir_lowering=False, debug=False)

    xso_d = nc.dram_tensor("x_so", [BPC, C, NHW], F16, kind="ExternalInput").ap()
    xc_d = nc.dram_tensor("x_c", [BPC, C, NHW], F16, kind="ExternalInput").ap()
    wso_d = nc.dram_tensor("wso", [128, OC1 * KC1 * 128], F16, kind="ExternalInput").ap()
    wc_d = nc.dram_tensor("wc", [128, OC1 * KC1 * 128], F16, kind="ExternalInput").ap()
    w1_d = nc.dram_tensor("w1", [128, OC2 * KC2 * 128], F16, kind="ExternalInput").ap()
    w2_d = nc.dram_tensor("w2", [128, OC2], F16, kind="ExternalInput").ap()
    bias_d = nc.dram_tensor("bias", [128, 2 * OC1 + OC2], F32, kind="ExternalInput").ap()
    out_d = nc.dram_tensor("out", [BPC * NN], F32, kind="ExternalOutput").ap()

    with tile.TileContext(nc) as tc:
        with (
            tc.tile_pool(name="wp", bufs=1) as wp,
            tc.tile_pool(name="xp", bufs=4) as xp,
            tc.tile_pool(name="fp", bufs=2) as fp,
            tc.tile_pool(name="hp", bufs=2) as hp,
            tc.tile_pool(name="ap", bufs=1) as ac,
            tc.tile_pool(name="ps1", bufs=4, space="PSUM") as ps1,
            tc.tile_pool(name="ps2", bufs=2, space="PSUM") as ps2,
            tc.tile_pool(name="ps3", bufs=2, space="PSUM") as ps3,
        ):
            # ---- tiny tensors on the gpsimd (SWDGE) ring ----
            # memset first: it gates the PE pre-warm matmuls.
            dummy_sb = wp.tile([128, 128], F16)
            nc.gpsimd.memset(dummy_sb[:], 0)
            bias_sb = wp.tile([128, 2 * OC1 + OC2], F32)
            nc.gpsimd.dma_start(bias_sb[:], bias_d[:])
            w2_sb = wp.tile([128, OC2], F16)
            nc.gpsimd.dma_start(w2_sb[:], w2_d[:])

            # ---- weights on the scalar (HWDGE, Activation) ring ----
            wso_sb = wp.tile([128, OC1 * KC1 * 128], F16)
            wso_t = wso_sb.rearrange("p (o r) -> p o r", o=OC1)
            wso_v = wso_d.rearrange("p (o r) -> p o r", o=OC1)
            for o in range(OC1):
                nc.scalar.dma_start(wso_t[:, o:o + 1, :], wso_v[:, o:o + 1, :])
            wc_sb = wp.tile([128, OC1 * KC1 * 128], F16)
            nc.scalar.dma_start(wc_sb[:], wc_d[:])
            w1_sb = wp.tile([128, OC2 * KC2 * 128], F16)
            nc.scalar.dma_start(w1_sb[:], w1_d[:])

            # ---- PE pre-warm: HAM flips to 2.4GHz after ~3.4us of
            # activity; burn the DMA lead-in on dummy matmuls so the real
            # stream runs warm almost immediately.
            wps = ps1.tile([128, MT], F32, tag="ps1")
            for i in range(16):
                nc.tensor.matmul(wps[:, 0:128], lhsT=dummy_sb[:],
                                 rhs=dummy_sb[:], start=True, stop=True)

            out_sb = ac.tile([1, BPC * NN], F32)
            bits_sb = ac.tile([1, BPC * NN], F32)

            import contextlib
            loop_cm = tc.For_i(0, loop, 1) if loop > 1 else contextlib.nullcontext()
            with loop_cm:
                _body(nc, tc, repeat, xso_d, xc_d, out_d,
                      wso_sb, wc_sb, w1_sb, w2_sb, bias_sb,
                      out_sb, bits_sb, xp, fp, hp, ps1, ps2, ps3, thresh)

    nc.compile()
    return nc


def _body(nc, tc, repeat, xso_d, xc_d, out_d,
          wso_sb, wc_sb, w1_sb, w2_sb, bias_sb,
          out_sb, bits_sb, xp, fp, hp, ps1, ps2, ps3, thresh):
    JS = KC1 // SUB
    for _rep in range(repeat):
        for b in range(BPC):
            # ---- x loads: sync HWDGE ring, in consumption order ----
            xso_sb = xp.tile([128, KC1 * NHW], F16, tag="xso")
            xso_t = xso_sb.rearrange("p (j m) -> p j m", j=KC1)
            xso_v = xso_d[b].rearrange("(p j) m -> p j m", p=128)
            for s in range(SUB):
                nc.sync.dma_start(xso_t[:, JS * s:JS * (s + 1), :],
                                  xso_v[:, JS * s:JS * (s + 1), :])
            xc_sb = xp.tile([128, KC1 * NHW], F16, tag="xc")
            xc_t = xc_sb.rearrange("p (j m) -> p j m", j=KC1)
            xc_v = xc_d[b].rearrange("(p j) m -> p j m", p=128)
            for s in range(SUB):
                nc.sync.dma_start(xc_t[:, JS * s:JS * (s + 1), :],
                                  xc_v[:, JS * s:JS * (s + 1), :])

            # ---- layer 1: f = lrelu(A @ x + a), fp16 out ----
            f_sb = fp.tile([128, 2 * OC1 * NHW], F16, tag="f")
            for br, (x_sb, w_sb, boff) in enumerate(
                ((xso_sb, wso_sb, 0), (xc_sb, wc_sb, OC1))
            ):
                for m in range(2):
                    for o in range(OC1):
                        ps = ps1.tile([128, MT], F32, tag="ps1")
                        for k in range(KC1):
                            nc.tensor.matmul(
                                ps[:],
                                lhsT=w_sb[:, (o * KC1 + k) * 128:
                                          (o * KC1 + k) * 128 + 128],
                                rhs=x_sb[:, k * NHW + m * MT:
                                         k * NHW + m * MT + MT],
                                start=(k == 0), stop=(k == KC1 - 1))
                        col = (br * OC1 + o) * NHW + m * MT
                        nc.scalar.activation(
                            f_sb[:, col:col + MT], ps[:],
                            mybir.ActivationFunctionType.Prelu,
                            bias=bias_sb[:, boff + o:boff + o + 1],
                            scale=1.0, alpha=SLOPE)

            # ---- layer 2: h = lrelu(A1 @ f + a1), fp16 out ----
            h_sb = hp.tile([128, OC2 * NHW], F16, tag="h")
            for m in range(2):
                for o in range(OC2):
                    ps = ps2.tile([128, MT], F32, tag="ps2")
                    for k in range(KC2):
                        nc.tensor.matmul(
                            ps[:],
                            lhsT=w1_sb[:, (o * KC2 + k) * 128:
                                       (o * KC2 + k) * 128 + 128],
                            rhs=f_sb[:, k * NHW + m * MT:
                                     k * NHW + m * MT + MT],
                            start=(k == 0), stop=(k == KC2 - 1))
                    col = o * NHW + m * MT
                    nc.scalar.activation(
                        h_sb[:, col:col + MT], ps[:],
                        mybir.ActivationFunctionType.Prelu,
                        bias=bias_sb[:, 2 * OC1 + o:2 * OC1 + o + 1],
                        scale=1.0, alpha=SLOPE)

            # ---- layer 3 + mean-reduce: y = W2 @ h ; sum 49-groups ----
            for m in range(2):
                ps = ps3.tile([1, MT], F32, tag="ps3")
                for q in range(OC2):
                    nc.tensor.matmul(
                        ps[:],
                        lhsT=w2_sb[:, q:q + 1],
                        rhs=h_sb[:, q * NHW + m * MT:
                                 q * NHW + m * MT + MT],
                        start=(q == 0), stop=(q == OC2 - 1))
                off = b * NN + m * (MT // HW)
                nc.vector.reduce_sum(
                    out_sb[0:1, off:off + MT // HW],
                    ps.rearrange("p (g x) -> p g x", x=HW),
                    axis=mybir.AxisListType.X)

        # ---- threshold: sigmoid(mean) > 0.5  <=>  sum > -49*b2 ----
        nc.vector.tensor_scalar(
            bits_sb[:], out_sb[:], float(thresh), None,
            mybir.AluOpType.is_gt)
        nc.sync.dma_start(out_d[:], bits_sb[0:1, :])


_CACHE = {}


def _get_nc(thresh, repeat=1, loop=1):
    key = (round(thresh, 9), repeat, loop)
    if key not in _CACHE:
        _CACHE[key] = build_bass(thresh, repeat, loop)
    return _CACHE[key]


def _prepare(inputs):
    """Fold params, cast x to fp16, build per-core input maps + nc."""
    wso, wc, w1, w2, bias, thresh = _fold_params(inputs)
    xso = np.asarray(inputs["x_so"], dtype=np.float32).reshape(
        B, C, NHW).astype(np.float16)
    xc = np.asarray(inputs["x_c"], dtype=np.float32).reshape(
        B, C, NHW).astype(np.float16)
    in_maps = []
    for i in range(N_CORES):
        in_maps.append({
            "x_so": xso[i * BPC:(i + 1) * BPC],
            "x_c": xc[i * BPC:(i + 1) * BPC],
            "wso": wso, "wc": wc, "w1": w1, "w2": w2, "bias": bias,
        })
    return _get_nc(thresh), in_maps


def kernel(**inputs):
    nc, in_maps = _prepare(inputs)
    res = run_bass_kernel_spmd(nc, in_maps, list(range(N_CORES)))
    out = np.concatenate([res.results[i]["out"].reshape(BPC, NN)
                          for i in range(N_CORES)], axis=0)
    return np.ascontiguousarray(out.reshape(B, NN, 1).astype(np.float32))


# revision 4
# speedup vs baseline: 1.1807x; 1.1807x over previous
"""Trainium2 Bass kernel for nn_ContextGatingSigmoidClassifier.

Math (eval mode):
  f_so = lrelu(W_so @ bn_so(x_so) + b_so)        x: [B,2048,N,H,W]
  f_c  = lrelu(W_c  @ bn_c(x_c)  + b_c)
  f    = concat -> bn1 -> W1 -> bn2 -> lrelu -> W2 -> mean(H,W) -> sigmoid > 0.5

All BatchNorms are eval-mode affine maps, so they fold into the adjacent
linear layers (done host-side in fp64). Final threshold:
  sigmoid(mean) > 0.5  <=>  sum_hw(W2 @ h) > -49*b2.

Device mapping: data-parallel over batch (4 per core, 8 cores), weights
replicated. x is cast fp32->fp16 host-side so the device reads half the
bytes; the kernel is tensor-engine-bound (1168 matmuls of N=294 ~146us
at 2.4GHz), so the whole game is keeping the PE stream dense:
  - ALL big transfers ride the sync HWDGE ring, which drains FIFO, in
    exact consumption order: wso o-block 0 (split in k-halves), x_so[0]
    (4 sub-DMAs), wso o-blocks 1-3, wc o-block 0, x_c[0], wc o-blocks
    1-3, w1, then x for batches 1-3. No ring contention, each transfer
    gets full SDMA bandwidth, and every weight block lands just before
    its first matmul.
  - L1 loops o-outer / m-inner so each o's weight block buys 4us of
    compute before the next one is needed.
  - tiny tensors (bias pack, w2) ride the gpsimd SWDGE ring in parallel.
  - a burst of dummy matmuls at t~7us pre-warms the PE HAM clock gate
    (cold PE runs at 1.2GHz for the first ~3.4us of activity).
Per batch element: x[b] is [2048, 588] (channels on SBUF partitions in
K-chunks of 128, positions on the free dim, 2 tiles of 294 = one PSUM
bank each). Channel->partition mapping is interleaved (partition p holds
channels 16p..16p+15) so every DMA descriptor is one contiguous
per-partition run; weights are permuted host-side to match (o-major).
"""

import numpy as np

import concourse.bass as bass  # noqa: F401
import concourse.tile as tile
from concourse import bacc, mybir
from concourse.bass_utils import run_bass_kernel_spmd

F16 = mybir.dt.float16
F32 = mybir.dt.float32

B, C, NN, HW = 32, 2048, 12, 49
NHW = NN * HW            # 588
N_CORES = 8
BPC = B // N_CORES       # 4 batch elements per core
MT = NHW // 2            # 294 columns = one PSUM bank of fp32
KC1 = C // 128           # 16 K-chunks, layer 1
OC1 = 512 // 128         # 4 output chunks, layer 1 (per branch)
KC2 = 1024 // 128        # 8 K-chunks, layer 2
OC2 = 256 // 128         # 2 output chunks, layer 2
SUB = 4                  # x sub-DMAs per tensor (4 k-chunks each)
EPS = 1e-5
SLOPE = 0.2


def _fold_params(d):
    """Fold BNs into linears, in fp64. Returns device-layout arrays."""
    g = {k: np.asarray(v, dtype=np.float64) for k, v in d.items()}

    def bn_st(p):
        s = g[f"{p}_g"] / np.sqrt(g[f"{p}_v"] + EPS)
        t = g[f"{p}_b"] - g[f"{p}_m"] * s
        return s, t

    s_so, t_so = bn_st("bn_so")
    s_c, t_c = bn_st("bn_c")
    s1, t1 = bn_st("bn1")
    s2, t2 = bn_st("bn2")

    A_so = g["W_so"] * s_so[None, :]                 # [512, 2048]
    a_so = g["W_so"] @ t_so + g["b_so"]              # [512]
    A_c = g["W_c"] * s_c[None, :]
    a_c = g["W_c"] @ t_c + g["b_c"]
    A1 = s2[:, None] * (g["W1"] * s1[None, :])       # [256, 1024]
    a1 = s2 * (g["W1"] @ t1 + g["b1"]) + t2          # [256]

    # layer-1 weights, o-major: W_dev[p, o, k, m] = A[128o+m, 16p+k]
    # (chunk k holds channel 16p+k at partition p, matching the x DMA
    # layout; o-major so each output block is one contiguous DMA).
    def l1_prep(A):  # [512, 2048] -> [128, OC1*KC1*128] fp16
        A4 = A.reshape(OC1, 128, 128, KC1)           # [o, m, p, k]
        return np.ascontiguousarray(
            A4.transpose(2, 0, 3, 1).reshape(128, OC1 * KC1 * 128)
        ).astype(np.float16)

    wso = l1_prep(A_so)
    wc = l1_prep(A_c)
    # layer-2: W1_dev[p, o, k, m] = A1[128o+m, 128k+p] (f stores channel
    # 128k+p at partition p of column-block k).
    A4 = A1.reshape(OC2, 128, KC2, 128)              # [o, m, k, p]
    w1 = np.ascontiguousarray(
        A4.transpose(3, 0, 2, 1).reshape(128, OC2 * KC2 * 128)
    ).astype(np.float16)
    w2 = np.ascontiguousarray(g["W2"].reshape(OC2, 128).T).astype(np.float16)
    # bias pack [128, 10] fp32: bso(4) | bc(4) | b1(2)
    bias = np.concatenate([
        a_so.reshape(OC1, 128).T, a_c.reshape(OC1, 128).T,
        a1.reshape(OC2, 128).T], axis=1)
    bias = np.ascontiguousarray(bias).astype(np.float32)
    thresh = float(-HW * g["b2"][0])
    return wso, wc, w1, w2, bias, thresh


def build_bass(thresh, repeat=1, loop=1):
    nc = bacc.Bacc("TRN2", target_bir_lowering=False, debug=False)

    xso_d = nc.dram_tensor("x_so", [BPC, C, NHW], F16, kind="ExternalInput").ap()
    xc_d = nc.dram_tensor("x_c", [BPC, C, NHW], F16, kind="ExternalInput").ap()
    wso_d = nc.dram_tensor("wso", [128, OC1 * KC1 * 128], F16, kind="ExternalInput").ap()
    wc_d = nc.dram_tensor("wc", [128, OC1 * KC1 * 128], F16, kind="ExternalInput").ap()
    w1_d = nc.dram_tensor("w1", [128, OC2 * KC2 * 128], F16, kind="ExternalInput").ap()
    w2_d = nc.dram_tensor("w2", [128, OC2], F16, kind="ExternalInput").ap()
    bias_d = nc.dram_tensor("bias", [128, 2 * OC1 + OC2], F32, kind="ExternalInput").ap()
    out_d = nc.dram_tensor("out", [BPC * NN], F32, kind="ExternalOutput").ap()

    with tile.TileContext(nc) as tc:
        with (
            tc.tile_pool(name="wp", bufs=1) as wp,
            tc.tile_pool(name="xp", bufs=4) as xp,
            tc.tile_pool(name="fp", bufs=2) as fp,
            tc.tile_pool(name="hp", bufs=2) as hp,
            tc.tile_pool(name="ap", bufs=1) as ac,
            tc.tile_pool(name="ps1", bufs=4, space="PSUM") as ps1,
            tc.tile_pool(name="ps2", bufs=2, space="PSUM") as ps2,
            tc.tile_pool(name="ps3", bufs=2, space="PSUM") as ps3,
        ):
            # ---- tiny tensors on the gpsimd (SWDGE) ring ----
            # memset first: it gates the PE pre-warm matmuls.
            dummy_sb = wp.tile([128, 128], F16)
            nc.gpsimd.memset(dummy_sb[:], 0)
            bias_sb = wp.tile([128, 2 * OC1 + OC2], F32)
            nc.gpsimd.dma_start(bias_sb[:], bias_d[:])
            w2_sb = wp.tile([128, OC2], F16)
            nc.gpsimd.dma_start(w2_sb[:], w2_d[:])

            # weight SBUF tiles (DMAs are issued inside _body, on the
            # sync ring, interleaved with x in consumption order)
            wso_sb = wp.tile([128, OC1 * KC1 * 128], F16)
            wc_sb = wp.tile([128, OC1 * KC1 * 128], F16)
            w1_sb = wp.tile([128, OC2 * KC2 * 128], F16)

            # ---- PE pre-warm: HAM flips to 2.4GHz after ~3.4us of
            # activity; burn the DMA lead-in on dummy matmuls so the
            # real stream runs warm almost immediately.
            wps = ps1.tile([128, MT], F32, tag="ps1")
            for i in range(16):
                nc.tensor.matmul(wps[:, 0:128], lhsT=dummy_sb[:],
                                 rhs=dummy_sb[:], start=True, stop=True)

            out_sb = ac.tile([1, BPC * NN], F32)
            bits_sb = ac.tile([1, BPC * NN], F32)

            import contextlib
            loop_cm = tc.For_i(0, loop, 1) if loop > 1 else contextlib.nullcontext()
            with loop_cm:
                _body(nc, tc, repeat, xso_d, xc_d, out_d,
                      (wso_d, wc_d, w1_d), wso_sb, wc_sb, w1_sb, w2_sb,
                      bias_sb, out_sb, bits_sb, xp, fp, hp, ps1, ps2, ps3,
                      thresh)

    nc.compile()
    return nc


def _body(nc, tc, repeat, xso_d, xc_d, out_d,
          weight_dram, wso_sb, wc_sb, w1_sb, w2_sb, bias_sb,
          out_sb, bits_sb, xp, fp, hp, ps1, ps2, ps3, thresh):
    wso_d, wc_d, w1_d = weight_dram
    JS = KC1 // SUB
    OB = KC1 * 128           # columns per o-block of a layer-1 weight

    def w_block(sb, dr, o, half=None):
        """sync-ring DMA of one o-block (optionally one k-half) of a
        layer-1 weight."""
        lo, hi = o * OB, (o + 1) * OB
        if half is not None:
            mid = lo + OB // 2
            lo, hi = (lo, mid) if half == 0 else (mid, hi)
        nc.sync.dma_start(sb[:, lo:hi], dr[:, lo:hi])

    def x_load(x_d, b, tag):
        x_sb = xp.tile([128, KC1 * NHW], F16, tag=tag)
        x_t = x_sb.rearrange("p (j m) -> p j m", j=KC1)
        x_v = x_d[b].rearrange("(p j) m -> p j m", p=128)
        for s in range(SUB):
            nc.sync.dma_start(x_t[:, JS * s:JS * (s + 1), :],
                              x_v[:, JS * s:JS * (s + 1), :])
        return x_sb

    for _rep in range(repeat):
        for b in range(BPC):
            first = _rep == 0 and b == 0
            if first:
                # consumption-ordered ring: wso o0 first (split so the
                # very first matmul starts ~1us earlier), then x_so[0].
                w_block(wso_sb, wso_d, 0, half=0)
                w_block(wso_sb, wso_d, 0, half=1)
            xso_sb = x_load(xso_d, b, "xso")
            if first:
                for o in range(1, OC1):
                    w_block(wso_sb, wso_d, o)
                w_block(wc_sb, wc_d, 0)
            xc_sb = x_load(xc_d, b, "xc")
            if first:
                for o in range(1, OC1):
                    w_block(wc_sb, wc_d, o)
                nc.sync.dma_start(w1_sb[:], w1_d[:])

            # ---- layer 1: f = lrelu(A @ x + a), fp16 out ----
            f_sb = fp.tile([128, 2 * OC1 * NHW], F16, tag="f")
            for br, (x_sb, w_sb, boff) in enumerate(
                ((xso_sb, wso_sb, 0), (xc_sb, wc_sb, OC1))
            ):
                for o in range(OC1):
                    for m in range(2):
                        ps = ps1.tile([128, MT], F32, tag="ps1")
                        for k in range(KC1):
                            nc.tensor.matmul(
                                ps[:],
                                lhsT=w_sb[:, (o * KC1 + k) * 128:
                                          (o * KC1 + k) * 128 + 128],
                                rhs=x_sb[:, k * NHW + m * MT:
                                         k * NHW + m * MT + MT],
                                start=(k == 0), stop=(k == KC1 - 1))
                        col = (br * OC1 + o) * NHW + m * MT
                        nc.scalar.activation(
                            f_sb[:, col:col + MT], ps[:],
                            mybir.ActivationFunctionType.Prelu,
                            bias=bias_sb[:, boff + o:boff + o + 1],
                            scale=1.0, alpha=SLOPE)

            # ---- layer 2: h = lrelu(A1 @ f + a1), fp16 out ----
            h_sb = hp.tile([128, OC2 * NHW], F16, tag="h")
            for m in range(2):
                for o in range(OC2):
                    ps = ps2.tile([128, MT], F32, tag="ps2")
                    for k in range(KC2):
                        nc.tensor.matmul(
                            ps[:],
                            lhsT=w1_sb[:, (o * KC2 + k) * 128:
                                       (o * KC2 + k) * 128 + 128],
                            rhs=f_sb[:, k * NHW + m * MT:
                                     k * NHW + m * MT + MT],
                            start=(k == 0), stop=(k == KC2 - 1))
                    col = o * NHW + m * MT
                    nc.scalar.activation(
                        h_sb[:, col:col + MT], ps[:],
                        mybir.ActivationFunctionType.Prelu,
                        bias=bias_sb[:, 2 * OC1 + o:2 * OC1 + o + 1],
                        scale=1.0, alpha=SLOPE)

            # ---- layer 3 + mean-reduce: y = W2 @ h ; sum 49-groups ----
            for m in range(2):
                ps = ps3.tile([1, MT], F32, tag="ps3")
                for q in range(OC2):
                    nc.tensor.matmul(
                        ps[:],
                        lhsT=w2_sb[:, q:q + 1],
                        rhs=h_sb[:, q * NHW + m * MT:
                                 q * NHW + m * MT + MT],
                        start=(q == 0), stop=(q == OC2 - 1))
                off = b * NN + m * (MT // HW)
                nc.vector.reduce_sum(
                    out_sb[0:1, off:off + MT // HW],
                    ps.rearrange("p (g x) -> p g x", x=HW),
                    axis=mybir.AxisListType.X)

        # ---- threshold: sigmoid(mean) > 0.5  <=>  sum > -49*b2 ----
        nc.vector.tensor_scalar(
            bits_sb[:], out_sb[:], float(thresh), None,
            mybir.AluOpType.is_gt)
        nc.sync.dma_start(out_d[:], bits_sb[0:1, :])


_CACHE = {}


def _get_nc(thresh, repeat=1, loop=1):
    key = (round(thresh, 9), repeat, loop)
    if key not in _CACHE:
        _CACHE[key] = build_bass(thresh, repeat, loop)
    return _CACHE[key]


def _prepare(inputs):
    """Fold params, cast x to fp16, build per-core input maps + nc."""
    wso, wc, w1, w2, bias, thresh = _fold_params(inputs)
    xso = np.asarray(inputs["x_so"], dtype=np.float32).reshape(
        B, C, NHW).astype(np.float16)
    xc = np.asarray(inputs["x_c"], dtype=np.float32).reshape(
        B, C, NHW).astype(np.float16)
    in_maps = []
    for i in range(N_CORES):
        in_maps.append({
            "x_so": xso[i * BPC:(i + 1) * BPC],
            "x_c": xc[i * BPC:(i + 1) * BPC],
            "wso": wso, "wc": wc, "w1": w1, "w2": w2, "bias": bias,
        })
    return _get_nc(thresh), in_maps


def kernel(**inputs):
    nc, in_maps = _prepare(inputs)
    res = run_bass_kernel_spmd(nc, in_maps, list(range(N_CORES)))
    out = np.concatenate([res.results[i]["out"].reshape(BPC, NN)
                          for i in range(N_CORES)], axis=0)
    return np.ascontiguousarray(out.reshape(B, NN, 1).astype(np.float32))
